# revision 10
# baseline (speedup 1.0000x reference)
"""Trainium2 Bass kernel for the e3nn-style GNN convolution layer.

kernel(**inputs) takes FULL (unsharded) numpy inputs and returns the FULL
[N, 160] float32 output.  Internally shards edges across 8 NeuronCores by
destination-node range, runs one SPMD Bass program, and reassembles on host.

Design:
  host prep   - fold all scalar normalizations into weights; x~ = node_input *
                node_attr; transpose node features (1o block c-major); sort
                edges by (dst-window, src-table-half), pad each half-group to
                a fixed number of 128-edge tiles (padding has edge_attr = 0 so
                its messages vanish).
  phase A     - per 128-node window: one fused [s | y] float32r matmul
                (self-connection s kept in SBUF, lin1 output y written bf16);
                AllGather replicates the y table across the 8 cores.
  edge phase  - per window: 2 dma_gather calls fetch y[src] rows (512B each)
                from the two table halves; per 128-edge tile: FC net (2 bf16
                matmuls + Silu), message build on DVE/ACT, and a one-hot
                selection matmul accumulating z in PSUM.
  node phase  - per window: transpose z (PE), lin2 (4 bf16 matmuls), add s,
                write the output slice.
"""

import math
from dataclasses import dataclass

import numpy as np
import ml_dtypes

import concourse.bacc as bacc
import concourse.bass as bass
import concourse.mybir as mybir
import concourse.tile as tile
from concourse.bass_utils import run_bass_kernel_spmd
from concourse.masks import make_identity

USE_ALLGATHER = True
BF16 = ml_dtypes.bfloat16
F32 = np.float32

MUL0 = 64
MUL1 = 32
FC_IN = 16
FC_H = 64
WN = 2 * MUL0 + 2 * MUL1  # 192 per-edge tp weights
D_IN = MUL0 + 3 * MUL1    # 160
DE = 256                  # padded y-table row elems (512 B in bf16)
D_MID = 4 * (MUL0 + MUL1) # 384 = [z0 (96) | z1_c0 | z1_c1 | z1_c2]
NUM_NEIGH = 10.0
C_S = math.sin(math.pi / 8.0)
C_X = math.cos(math.pi / 8.0)
P = 128


@dataclass(frozen=True)
class Cfg:
    n: int
    n_cores: int
    npc: int          # nodes per core
    wpc: int          # 128-node windows per core
    npad: int         # wpc * 128
    th: int           # tiles per (window, table-half)
    use_allgather: bool = True


def _to_cmajor(x_uc):
    s = x_uc.shape[:-1]
    return x_uc.reshape(*s, MUL1, 3).swapaxes(-1, -2).reshape(*s, 96)


def _from_cmajor(x_cu):
    s = x_cu.shape[:-1]
    return x_cu.reshape(*s, 3, MUL1).swapaxes(-1, -2).reshape(*s, 96)


# ---------------------------------------------------------------- host prep

def _prep(inputs, n_cores=8):
    node_input = np.asarray(inputs["node_input"], F32)
    node_attr = np.asarray(inputs["node_attr"], F32)
    edge_src = np.asarray(inputs["edge_src"]).astype(np.int64)
    edge_dst = np.asarray(inputs["edge_dst"]).astype(np.int64)
    edge_attr = np.asarray(inputs["edge_attr"], F32)
    ele = np.asarray(inputs["edge_length_embedded"], F32)

    n = node_input.shape[0]
    e = edge_src.shape[0]
    assert n % n_cores == 0
    npc = n // n_cores
    wpc = (npc + P - 1) // P
    npad = wpc * P
    ntab = n_cores * npad
    half = ntab // 2
    assert half <= 32767 and half % npad == 0

    inv0 = 1.0 / math.sqrt(MUL0)
    inv1 = 1.0 / math.sqrt(MUL1)
    invm = 1.0 / math.sqrt(MUL0 + MUL1)
    invnb = 1.0 / math.sqrt(NUM_NEIGH)

    x = node_input * node_attr
    xT = np.concatenate([x[:, :MUL0], _to_cmajor(x[:, MUL0:])], axis=1).T  # [160,n]
    xT = np.ascontiguousarray(xT, F32)

    W_sc0 = np.asarray(inputs["W_sc0"], F32) * (inv0 * C_S)
    W_sc1 = np.asarray(inputs["W_sc1"], F32) * (inv1 * C_S)
    W_l1_0 = np.asarray(inputs["W_l1_0"], F32) * inv0
    W_l1_1 = np.asarray(inputs["W_l1_1"], F32) * inv1
    fc_W1 = np.asarray(inputs["fc_W1"], F32) * (1.0 / math.sqrt(FC_IN))
    fc_W2 = np.asarray(inputs["fc_W2"], F32) * (1.0 / math.sqrt(FC_H))
    obase = invm * C_X * invnb
    W_l2_0 = np.asarray(inputs["W_l2_0"], F32) * obase
    W_l2_0 = W_l2_0.copy()
    W_l2_0[MUL0:, :] *= 1.0 / math.sqrt(3.0)
    W_l2_1 = np.asarray(inputs["W_l2_1"], F32) * obase

    def blockdiag(*ms):
        rows = sum(m.shape[0] for m in ms)
        cols = sum(m.shape[1] for m in ms)
        out = np.zeros((rows, cols), F32)
        r = c = 0
        for m in ms:
            out[r:r + m.shape[0], c:c + m.shape[1]] = m
            r += m.shape[0]
            c += m.shape[1]
        return out

    Wsc_big = blockdiag(W_sc0, W_sc1, W_sc1, W_sc1)
    Wl1_big = blockdiag(W_l1_0, W_l1_1, W_l1_1, W_l1_1)
    AB = np.ascontiguousarray(np.concatenate([Wsc_big, Wl1_big], axis=1), F32)

    # ---- edge sharding: (dst-window, src-half) groups
    core = edge_dst // npc
    local = edge_dst - core * npc
    win = local // P
    ldst = (local - win * P).astype(F32)
    src_remap = (edge_src // npc) * npad + (edge_src % npc)
    hbit = (src_remap >= half).astype(np.int64)
    g2 = (core * wpc + win) * 2 + hbit              # (window, half) group id
    order = np.argsort(g2, kind="stable")
    cnt2 = np.bincount(g2, minlength=n_cores * wpc * 2)
    th = max(1, int((cnt2.max() + P - 1) // P))     # tiles per half-group
    ni = th * P                                     # slots per half-group
    epw = 2 * ni                                    # edge slots per window
    tt = 2 * th                                     # tiles per window

    starts = np.zeros(n_cores * wpc * 2, np.int64)
    starts[1:] = np.cumsum(cnt2)[:-1]
    j_within = np.arange(e) - starts[g2[order]]
    dest = g2[order] * ni + j_within                # flat padded slot

    flat = n_cores * wpc * epw
    A_ = np.zeros((flat, 4), F32)
    A_[dest] = edge_attr[order]
    IDX = np.zeros(flat, np.int16)
    IDX[dest] = (src_remap[order] - hbit[order] * half).astype(np.int16)
    L_ = np.zeros(flat, F32)
    L_[dest] = ldst[order]
    E_ = np.zeros((flat, FC_IN), F32)
    E_[dest] = ele[order]

    A_ = A_.reshape(n_cores, wpc, tt, P, 4).transpose(0, 1, 3, 2, 4)
    attr_p = np.ascontiguousarray(A_.reshape(n_cores, wpc, P, tt * 4), F32)
    ldst_p = np.ascontiguousarray(
        L_.reshape(n_cores, wpc, tt, P).transpose(0, 1, 3, 2), BF16)
    eleT_p = np.ascontiguousarray(
        E_.reshape(n_cores, wpc * tt * P, FC_IN).transpose(0, 2, 1), BF16)
    # idx wrapped for dma_gather: j -> (j%16, j//16), replicated over 8 groups
    ni16 = ni // 16
    IW = IDX.reshape(n_cores, wpc, 2, ni16, 16).swapaxes(3, 4)  # [c,w,h,16,ni16]
    idx_p = np.ascontiguousarray(
        np.broadcast_to(IW[:, :, :, None, :, :],
                        (n_cores, wpc, 2, 8, 16, ni16))
        .reshape(n_cores, wpc, 2, P, ni16))

    xT_pad = np.zeros((n_cores, D_IN, npad), F32)
    for k in range(n_cores):
        xT_pad[k, :, :npc] = xT[:, k * npc:(k + 1) * npc]
    xTbf_full = np.ascontiguousarray(
        xT_pad.transpose(1, 0, 2).reshape(D_IN, ntab), BF16)

    cfg = Cfg(n=n, n_cores=n_cores, npc=npc, wpc=wpc, npad=npad, th=th,
              use_allgather=USE_ALLGATHER)

    in_maps = []
    for k in range(n_cores):
        m = {
            "xTf": np.ascontiguousarray(xT_pad[k]),
            "AB_w": AB,
            "eleT": eleT_p[k],
            "attr_p": attr_p[k],
            "idx_p": idx_p[k],
            "ldst_p": ldst_p[k],
            "fcW1": np.ascontiguousarray(fc_W1, BF16),
            "fcW2": np.ascontiguousarray(fc_W2, BF16),
            "Wl2_0c": np.ascontiguousarray(W_l2_0, BF16),
            "Wl2_1c": np.ascontiguousarray(W_l2_1, BF16),
        }
        if not cfg.use_allgather:
            m["xTbf"] = xTbf_full
            m["Wl1b"] = np.ascontiguousarray(Wl1_big, BF16)
        in_maps.append(m)
    return cfg, in_maps, node_attr


# ---------------------------------------------------------------- device program

_PROG_CACHE = {}


def _build(cfg: Cfg):
    if cfg in _PROG_CACHE:
        return _PROG_CACHE[cfg]

    th, wpc, npad = cfg.th, cfg.wpc, cfg.npad
    tt = 2 * th
    ni = th * P
    ni16 = ni // 16
    ep = wpc * tt * P
    ntab = cfg.n_cores * npad
    half = ntab // 2
    bf = mybir.dt.bfloat16
    f32 = mybir.dt.float32
    f32r = mybir.dt.float32r
    i16 = mybir.dt.int16

    nc = bacc.Bacc("TRN2", target_bir_lowering=False, debug=False,
                   num_devices=cfg.n_cores)

    xTf = nc.dram_tensor("xTf", [D_IN, npad], f32r, kind="ExternalInput")
    AB_w = nc.dram_tensor("AB_w", [D_IN, 320], f32r, kind="ExternalInput")
    eleT = nc.dram_tensor("eleT", [FC_IN, ep], bf, kind="ExternalInput")
    attr_p = nc.dram_tensor("attr_p", [wpc, P, 4 * tt], f32, kind="ExternalInput")
    idx_p = nc.dram_tensor("idx_p", [wpc, 2, P, ni16], i16, kind="ExternalInput")
    ldst_p = nc.dram_tensor("ldst_p", [wpc, P, tt], bf, kind="ExternalInput")
    fcW1 = nc.dram_tensor("fcW1", [FC_IN, FC_H], bf, kind="ExternalInput")
    fcW2 = nc.dram_tensor("fcW2", [FC_H, WN], bf, kind="ExternalInput")
    Wl2_0c = nc.dram_tensor("Wl2_0c", [96, MUL0], bf, kind="ExternalInput")
    Wl2_1c = nc.dram_tensor("Wl2_1c", [96, MUL1], bf, kind="ExternalInput")
    out_d = nc.dram_tensor("out", [npad, D_IN], f32, kind="ExternalOutput")

    y_table = nc.dram_tensor("y_table", [ntab, DE], bf, addr_space="Shared")
    if cfg.use_allgather:
        y_bounce = nc.dram_tensor("y_bounce", [npad, DE], bf)
    else:
        xTbf = nc.dram_tensor("xTbf", [D_IN, ntab], bf, kind="ExternalInput")
        Wl1b = nc.dram_tensor("Wl1b", [D_IN, D_IN], bf, kind="ExternalInput")

    with tile.TileContext(nc) as tc:
        with (
            tc.tile_pool(name="const", bufs=1) as cpool,
            tc.tile_pool(name="work", bufs=2) as wp,
            tc.tile_pool(name="psA", bufs=2, space="PSUM") as psA,
            tc.tile_pool(name="we", bufs=2) as we,
            tc.tile_pool(name="msgp", bufs=3) as mp,
            tc.tile_pool(name="psE", bufs=2, space="PSUM") as psE,
            tc.tile_pool(name="psZ", bufs=2, space="PSUM") as psZ,
        ):
            # ---- constants
            iota_i = cpool.tile([P, P], mybir.dt.int32)
            nc.gpsimd.iota(iota_i[:], pattern=[[1, P]], base=0, channel_multiplier=0)
            iota_bf = cpool.tile([P, P], bf)
            nc.vector.tensor_copy(out=iota_bf[:], in_=iota_i[:])
            ident = cpool.tile([P, P], bf)
            make_identity(nc, ident[:])

            fcW1_sb = cpool.tile([FC_IN, FC_H], bf)
            nc.sync.dma_start(out=fcW1_sb[:], in_=fcW1[:, :])
            fcW2_sb = cpool.tile([FC_H, WN], bf)
            nc.sync.dma_start(out=fcW2_sb[:], in_=fcW2[:, :])
            Wl20_sb = cpool.tile([96, MUL0], bf)
            nc.sync.dma_start(out=Wl20_sb[:], in_=Wl2_0c[:, :])
            Wl21_sb = cpool.tile([96, MUL1], bf)
            nc.sync.dma_start(out=Wl21_sb[:], in_=Wl2_1c[:, :])

            AB0 = cpool.tile([P, 320], f32r)
            nc.sync.dma_start(out=AB0[:], in_=AB_w[0:P, :])
            AB1 = cpool.tile([D_IN - P, 320], f32r)
            nc.sync.dma_start(out=AB1[:], in_=AB_w[P:D_IN, :])
            if not cfg.use_allgather:
                Wl1b0 = cpool.tile([P, D_IN], bf)
                nc.sync.dma_start(out=Wl1b0[:], in_=Wl1b[0:P, :])
                Wl1b1 = cpool.tile([D_IN - P, D_IN], bf)
                nc.sync.dma_start(out=Wl1b1[:], in_=Wl1b[P:D_IN, :])

            s_store = cpool.tile([P, wpc * D_IN], f32)

            # ---- phase A: s (self-connection) + local y slice
            for w in range(wpc):
                xa = wp.tile([P, P], f32r, tag="xa")
                nc.sync.dma_start(out=xa[:], in_=xTf[0:P, w * P:(w + 1) * P])
                xb = wp.tile([D_IN - P, P], f32r, tag="xb")
                nc.sync.dma_start(out=xb[:], in_=xTf[P:D_IN, w * P:(w + 1) * P])
                sy = psA.tile([P, 320], f32, tag="sy")
                nc.tensor.matmul(out=sy[:], lhsT=xa[:], rhs=AB0[:],
                                 start=True, stop=False)
                nc.tensor.matmul(out=sy[:], lhsT=xb[:], rhs=AB1[:],
                                 start=False, stop=True)
                nc.vector.tensor_copy(out=s_store[:, w * D_IN:(w + 1) * D_IN],
                                      in_=sy[:, 0:D_IN])
                y_sb = wp.tile([P, D_IN], bf, tag="ysb")
                nc.vector.tensor_copy(out=y_sb[:], in_=sy[:, D_IN:2 * D_IN])
                if cfg.use_allgather:
                    nc.sync.dma_start(out=y_bounce[w * P:(w + 1) * P, 0:D_IN],
                                      in_=y_sb[:])
                else:
                    nc.sync.dma_start(out=y_table[w * P:(w + 1) * P, 0:D_IN],
                                      in_=y_sb[:])

            if cfg.use_allgather:
                nc.gpsimd.collective_compute(
                    "AllGather",
                    mybir.AluOpType.bypass,
                    replica_groups=[list(range(cfg.n_cores))],
                    ins=[y_bounce[:, :]],
                    outs=[y_table[:, :]],
                )
            else:
                gwc = cfg.n_cores * wpc
                for g in range(gwc):
                    xab = wp.tile([P, P], bf, tag="xab")
                    nc.sync.dma_start(out=xab[:], in_=xTbf[0:P, g * P:(g + 1) * P])
                    xbb = wp.tile([D_IN - P, P], bf, tag="xbb")
                    nc.sync.dma_start(out=xbb[:], in_=xTbf[P:D_IN, g * P:(g + 1) * P])
                    yp_full = psA.tile([P, 320], f32, tag="sy")
                    yp = yp_full[:, 0:D_IN]
                    nc.tensor.matmul(out=yp, lhsT=xab[:], rhs=Wl1b0[:],
                                     start=True, stop=False)
                    nc.tensor.matmul(out=yp, lhsT=xbb[:], rhs=Wl1b1[:],
                                     start=False, stop=True)
                    yb2 = wp.tile([P, D_IN], bf, tag="ysb")
                    nc.vector.tensor_copy(out=yb2[:], in_=yp)
                    nc.sync.dma_start(out=y_table[g * P:(g + 1) * P, 0:D_IN],
                                      in_=yb2[:])

            # ---- edge + node phases
            MU = mybir.AluOpType.mult
            AD = mybir.AluOpType.add
            EQ = mybir.AluOpType.is_equal
            for w in range(wpc):
                idx_sb = we.tile([P, 2 * ni16], i16, tag="idx")
                nc.sync.dma_start(out=idx_sb[:, 0:ni16], in_=idx_p[w, 0, :, :])
                nc.sync.dma_start(out=idx_sb[:, ni16:2 * ni16], in_=idx_p[w, 1, :, :])
                ldst_sb = we.tile([P, tt], bf, tag="ldst")
                nc.sync.dma_start(out=ldst_sb[:], in_=ldst_p[w, :, :])
                at_sb = we.tile([P, 4 * tt], f32, tag="attr")
                nc.sync.dma_start(out=at_sb[:], in_=attr_p[w, :, :])
                el_sb = we.tile([FC_IN, tt * P], bf, tag="ele")
                nc.sync.dma_start(out=el_sb[:],
                                  in_=eleT[:, w * tt * P:(w + 1) * tt * P])
                ys_lo = we.tile([P, th * DE], bf, tag="yslo")
                nc.gpsimd.dma_gather(
                    out_ap=ys_lo[:].rearrange("p (t f) -> p t f", f=DE),
                    in_ap=y_table[0:half, :],
                    idxs_ap=idx_sb[:, 0:ni16],
                    num_idxs=ni, num_idxs_reg=ni, elem_size=DE,
                    single_packet=False)
                ys_hi = we.tile([P, th * DE], bf, tag="yshi")
                nc.gpsimd.dma_gather(
                    out_ap=ys_hi[:].rearrange("p (t f) -> p t f", f=DE),
                    in_ap=y_table[half:ntab, :],
                    idxs_ap=idx_sb[:, ni16:2 * ni16],
                    num_idxs=ni, num_idxs_reg=ni, elem_size=DE,
                    single_packet=False)

                z_ps = psZ.tile([P, D_MID], f32, tag="z")
                for t in range(tt):
                    ys_half = ys_lo if t < th else ys_hi
                    tb = t % th
                    hT_ps = psE.tile([FC_H, P], f32, tag="ht")
                    nc.tensor.matmul(out=hT_ps[:], lhsT=fcW1_sb[:],
                                     rhs=el_sb[:, t * P:(t + 1) * P],
                                     start=True, stop=True)
                    hT_sb = mp.tile([FC_H, P], bf, tag="hts")
                    nc.scalar.activation(out=hT_sb[:], in_=hT_ps[:],
                                         func=mybir.ActivationFunctionType.Silu)
                    w_ps = psE.tile([P, WN], f32, tag="w")
                    nc.tensor.matmul(out=w_ps[:], lhsT=hT_sb[:], rhs=fcW2_sb[:],
                                     start=True, stop=True)

                    S_sb = mp.tile([P, P], bf, tag="S")
                    nc.vector.tensor_tensor(
                        out=S_sb[:],
                        in0=ldst_sb[:, t:t + 1].to_broadcast([P, P]),
                        in1=iota_bf[:],
                        op=EQ)

                    yb = tb * DE
                    ys0 = ys_half[:, yb:yb + MUL0]
                    ys1 = [ys_half[:, yb + MUL0 + c * MUL1:yb + MUL0 + (c + 1) * MUL1]
                           for c in range(3)]
                    e0 = at_sb[:, 4 * t:4 * t + 1]
                    e1 = [at_sb[:, 4 * t + 1 + c:4 * t + 2 + c] for c in range(3)]
                    w_a = w_ps[:, 0:MUL0]
                    w_b = w_ps[:, MUL0:2 * MUL0]
                    w_c = w_ps[:, 2 * MUL0:2 * MUL0 + MUL1]
                    w_d = w_ps[:, 2 * MUL0 + MUL1:WN]

                    msg = mp.tile([P, D_MID], bf, tag="msg")
                    ta = mp.tile([P, MUL0], bf, tag="ta")
                    nc.vector.tensor_tensor(out=ta[:], in0=w_a, in1=ys0, op=MU)
                    nc.vector.tensor_scalar(out=msg[:, 0:MUL0], in0=ta[:],
                                            scalar1=e0, scalar2=None, op0=MU)
                    d0 = mp.tile([P, MUL1], bf, tag="d0")
                    nc.vector.tensor_scalar(out=d0[:], in0=ys1[0], scalar1=e1[0],
                                            scalar2=None, op0=MU)
                    d1 = mp.tile([P, MUL1], bf, tag="d1")
                    nc.vector.tensor_scalar(out=d1[:], in0=ys1[1], scalar1=e1[1],
                                            scalar2=None, op0=MU)
                    d2 = mp.tile([P, MUL1], bf, tag="d2")
                    nc.vector.tensor_scalar(out=d2[:], in0=ys1[2], scalar1=e1[2],
                                            scalar2=None, op0=MU)
                    nc.vector.tensor_tensor(out=d0[:], in0=d0[:], in1=d1[:], op=AD)
                    nc.vector.tensor_tensor(out=d0[:], in0=d0[:], in1=d2[:], op=AD)
                    nc.vector.tensor_tensor(out=msg[:, MUL0:96], in0=d0[:],
                                            in1=w_d, op=MU)
                    t2 = mp.tile([P, MUL0], bf, tag="t2")
                    nc.vector.tensor_tensor(out=t2[:], in0=w_b, in1=ys0, op=MU)
                    t3 = mp.tile([P, MUL1], bf, tag="t3")
                    nc.vector.tensor_scalar(out=t3[:], in0=w_c, scalar1=e0,
                                            scalar2=None, op0=MU)
                    for c in range(3):
                        base = 96 + c * 96
                        nc.scalar.activation(out=msg[:, base:base + MUL0], in_=t2[:],
                                             func=mybir.ActivationFunctionType.Copy,
                                             scale=e1[c])
                        nc.vector.tensor_tensor(out=msg[:, base + MUL0:base + 96],
                                                in0=t3[:], in1=ys1[c], op=MU)

                    nc.tensor.matmul(out=z_ps[:], lhsT=S_sb[:], rhs=msg[:],
                                     start=(t == 0), stop=(t == tt - 1))

                # ---- node phase for this window
                z_sb = mp.tile([P, D_MID], bf, tag="zsb")
                nc.vector.tensor_copy(out=z_sb[:], in_=z_ps[:])
                o_ps = psE.tile([P, D_IN], f32, tag="w")
                for b in range(4):
                    zT_ps = psE.tile([96, P], bf, tag="ht")
                    nc.tensor.transpose(out=zT_ps[:], in_=z_sb[:, b * 96:(b + 1) * 96],
                                        identity=ident[:])
                    zT_sb = mp.tile([96, P], bf, tag="zts")
                    nc.vector.tensor_copy(out=zT_sb[:], in_=zT_ps[:])
                    if b == 0:
                        nc.tensor.matmul(out=o_ps[:, 0:MUL0], lhsT=zT_sb[:],
                                         rhs=Wl20_sb[:], start=True, stop=True)
                    else:
                        c = b - 1
                        nc.tensor.matmul(
                            out=o_ps[:, MUL0 + c * MUL1:MUL0 + (c + 1) * MUL1],
                            lhsT=zT_sb[:], rhs=Wl21_sb[:], start=True, stop=True)
                out_sb = mp.tile([P, D_IN], f32, tag="outsb")
                nc.vector.tensor_tensor(out=out_sb[:], in0=o_ps[:],
                                        in1=s_store[:, w * D_IN:(w + 1) * D_IN],
                                        op=AD)
                nc.sync.dma_start(out=out_d[w * P:(w + 1) * P, :], in_=out_sb[:])

    nc.compile()
    _PROG_CACHE[cfg] = nc
    return nc


# ---------------------------------------------------------------- entry point

def _assemble(cfg: Cfg, results, node_attr):
    outs = [results[k]["out"][:cfg.npc] for k in range(cfg.n_cores)]
    o = np.concatenate(outs, axis=0).astype(F32)
    o = np.concatenate([o[:, :MUL0], _from_cmajor(o[:, MUL0:])], axis=1)
    return o * node_attr


def kernel(**inputs):
    cfg, in_maps, node_attr = _prep(inputs, n_cores=8)
    nc = _build(cfg)
    res = run_bass_kernel_spmd(nc, in_maps, core_ids=list(range(cfg.n_cores)))
    return _assemble(cfg, res.results, node_attr)
